# revision 1
# baseline (speedup 1.0000x reference)
"""CTC prefix-scorer kernel for 8 Trainium2 NeuronCores (Bass/Tile).

Math: the reference's scan collapses because gamma_n_g is statically NEG_INF,
so phi_t == B[t-1] (cumsum of blank log-probs) and the n/b carries are dead.
The output is
    score[j] = logsumexp_{t=9..T-1}( B[t-1] - lse[t] + ctc_prob[t, c[j]] )
    score[j] = B[T-1]                       if c[j] == EOS (==1)
    out      = score.reshape(N, ctc_beam)
which depends on j only through c[j], so we compute a per-vocab table S[v]
on-device and gather on host.

Sharding: vocab axis (32000) split into 8 x 4000 column slices.
  Launch 1: per-core partial row-sums of exp(x) over its vocab slice
            (ACT Exp with fused accum_out). Host combines -> lse, B, w.
  Launch 2: per-core colsum[v] = sum_t exp(x[t,v] + (w[t]-K)) via ACT Exp
            with per-partition bias, then TensorE ones-matmul reduction over
            the t (partition) axis accumulating in PSUM.
Host: S = K + log(colsum); score = S[c]; EOS override; reshape.

Hardware note: the ACT queue has a single semaphore-wait slot, so the kernels
are arranged so every Activation depends on exactly one cross-engine event:
t-blocks overlap (last block starts at T-128) instead of using partial
partitions + memset, each t-block gets its own live e-tile (no PE->ACT WAR),
and a dummy ACT copy absorbs the wv-load DMA semaphore up front.
"""

import os
import numpy as np

# ---- problem constants (hardcoded per contract) ----
T = 1500
V = 32000
N_BATCH = 8
CTC_BEAM = 2048
N_CORES = 8
VSLICE = V // N_CORES  # 4000
GLEN = 9               # U-1, static in the reference
EOS = 1
BLANK_COL = V - 1
NEG_BIG = np.float32(-1e30)

TBLK = 128
NTB = 12
# block starts; the last block overlaps the previous one (rows 1372..1499),
# duplicated rows are suppressed via -1e30 bias entries in launch 2 and by
# identical-overwrite merging in launch 1.
ST = [min(tb * TBLK, T - TBLK) for tb in range(NTB)]

# matmul column chunking of the 4000-wide vocab slice (PSUM free dim <= 512)
CHUNKS = [512] * 7 + [416]

_CACHE: dict = {}
LAST_EXEC_TIMES: list = []


def _limit_dma_sem_lanes():
    """Walrus on this toolchain accepts very few semaphore waits per
    instruction; the Tile kernel-tail drain waits on every DMA sem lane the
    kernel touched. All our DMAs go through one SWDGE queue / one issuing
    HWDGE engine, whose completions are FIFO per ring, so tracking them on a
    single counting semaphore per family is sound and keeps the drain's wait
    list tiny."""
    import concourse.tile_sem_assignment as tsa

    tsa.NUM_SWDGE_GLOBAL_SEMS = 1
    tsa.NUM_HWDGE_SEMS = 1


def _patch_drain_split():
    """The kernel-tail drain waits on every logical proc the kernel used, but
    walrus here accepts a single semaphore wait per instruction. Split the
    drain's wait list across a chain of single-wait drains on the same sync
    queue (executed in order, so the union of conditions is preserved)."""
    import bass_rust
    import concourse.tile as tile_mod
    from concourse.vector_clock import ScopedClock

    if getattr(tile_mod.TileContext, "_drain_split_patched", False):
        return

    def _drain_and_barrier(self, tick_clock, wait_clock):
        # drain on the ACT ring: its queue has already observed most procs
        # (exp waits cover the DMA lanes, copies cover PE), so the wait list
        # usually collapses to a single lane wait
        drain_inst = self.nc.scalar.drain()
        wait_clock.add_sem_waits(
            drain_inst.ins, ScopedClock({None: tick_clock.global_clock})
        )
        si = drain_inst.ins.sync_info
        waits = list(si.on_wait) if si is not None else []
        if len(waits) > 1:
            drain_inst.ins.sync_info = bass_rust.SyncInfo(
                on_wait=[waits[0]], on_update=list(si.on_update)
            )
            for wt in waits[1:]:
                extra = self.nc.sync.drain()
                extra.ins.sync_info = bass_rust.SyncInfo(
                    on_wait=[wt], on_update=[]
                )

        self.nc.all_engine_barrier()
        assert self.sems is not None
        popped = self.nc._tile_sem_poison_stack.pop()
        assert popped is self._sem_poison
        self.nc.clear_and_free_semaphores(list(self.sems.allocated().values()))
        self.nc.all_engine_barrier()

    tile_mod.TileContext._drain_and_barrier = _drain_and_barrier
    tile_mod.TileContext._drain_split_patched = True


def _build_rowsum_nc():
    """Launch-1 program: x (T, VSLICE) bf16 -> rowsum (128, NTB) f32 where
    rowsum[p, tb] = sum_v exp(x[ST[tb] + p, v])."""
    import concourse.bass as bass
    import concourse.tile as tile
    from concourse import mybir

    _limit_dma_sem_lanes()
    _patch_drain_split()
    nc = bass.Bass()
    x = nc.dram_tensor("x", [T, VSLICE], mybir.dt.bfloat16, kind="ExternalInput")
    rowsum = nc.dram_tensor(
        "rowsum", [TBLK, NTB], mybir.dt.float32, kind="ExternalOutput"
    )

    with tile.TileContext(nc) as tc:
        with tc.tile_pool(name="sing", bufs=1) as sing:
            partials = sing.tile([TBLK, NTB], mybir.dt.float32)
            # every instruction may carry at most ONE semaphore wait on this
            # toolchain, so each t-block gets its own live tile (no buffer
            # reuse -> DMAs wait on nothing, each ACT waits only on its DMA).
            # exp runs in-place on the bf16 tile; the f32 row-sum comes out
            # through accum_out.
            for tb in range(NTB):
                et = sing.tile(
                    [TBLK, VSLICE], mybir.dt.bfloat16, name=f"et{tb}", tag=f"et{tb}"
                )
                # alternate SWDGE/HWDGE ring rows: the ~2us completion-receipt
                # stall after each DMA's sem-update descriptor then overlaps
                # the other ring's data packets (SDMA engines round-robin
                # between ring rows at packet granularity)
                eng = nc.sync if tb % 2 == 0 else nc.gpsimd
                eng.dma_start(out=et, in_=x[ST[tb] : ST[tb] + TBLK, :])
                nc.scalar.activation(
                    out=et,
                    in_=et,
                    func=mybir.ActivationFunctionType.Exp,
                    accum_out=partials[:, tb : tb + 1],
                )
            # on the ACT ring: the partials dependency is implicit in
            # queue program order, so only the lane-window wait remains
            nc.scalar.dma_start(out=rowsum[:, :], in_=partials)
    return nc


def _build_score_nc():
    """Launch-2 program: x (T, VSLICE) bf16, wv (128, NTB) f32 (= w[t]-K per
    t-block column, -1e30 on invalid/duplicated t) -> colsum (1, VSLICE) f32
    with colsum[v] = sum_t exp(x[t, v] + wv[t])."""
    import concourse.bass as bass
    import concourse.tile as tile
    from concourse import mybir

    _limit_dma_sem_lanes()
    _patch_drain_split()
    nc = bass.Bass()
    x = nc.dram_tensor("x", [T, VSLICE], mybir.dt.bfloat16, kind="ExternalInput")
    wv = nc.dram_tensor("wv", [TBLK, NTB], mybir.dt.float32, kind="ExternalInput")
    colsum = nc.dram_tensor(
        "colsum", [1, VSLICE], mybir.dt.float32, kind="ExternalOutput"
    )

    with tile.TileContext(nc) as tc:
        with (
            tc.tile_pool(name="sing", bufs=1) as sing,
            tc.tile_pool(name="psp", space="PSUM", bufs=1) as psp,
            tc.tile_pool(name="outp", bufs=1) as outp,
        ):
            wv_sb = sing.tile([TBLK, NTB], mybir.dt.float32)
            nc.gpsimd.dma_start(out=wv_sb, in_=wv[:, :])
            # absorb the wv DMA semaphore into ACT's observed clock so the
            # per-block Exp below carries only its own x-load wait
            wv_probe = sing.tile([TBLK, 1], mybir.dt.float32)
            nc.scalar.copy(out=wv_probe, in_=wv_sb[:, 0:1])

            ones = nc.const_aps.tensor(1.0, (TBLK, 1), mybir.dt.bfloat16)

            ps = [
                psp.tile([1, 512], mybir.dt.float32, name=f"ps{ch}", tag=f"ps{ch}")
                for ch in range(len(CHUNKS))
            ]

            for tb in range(NTB):
                et = sing.tile(
                    [TBLK, VSLICE], mybir.dt.bfloat16, name=f"et{tb}", tag=f"et{tb}"
                )
                eng = nc.sync if tb % 2 == 0 else nc.gpsimd
                eng.dma_start(out=et, in_=x[ST[tb] : ST[tb] + TBLK, :])
                nc.scalar.activation(
                    out=et,
                    in_=et,
                    func=mybir.ActivationFunctionType.Exp,
                    bias=wv_sb[:, tb : tb + 1],
                    scale=1.0,
                )
                s = 0
                for ch, cw in enumerate(CHUNKS):
                    nc.tensor.matmul(
                        ps[ch][:, :cw],
                        ones,
                        et[:, s : s + cw],
                        start=(tb == 0),
                        stop=(tb == NTB - 1),
                    )
                    s += cw

            out_sb = outp.tile([1, VSLICE], mybir.dt.float32)
            s = 0
            for ch, cw in enumerate(CHUNKS):
                nc.scalar.copy(out=out_sb[:, s : s + cw], in_=ps[ch][:, :cw])
                s += cw
            nc.scalar.dma_start(out=colsum[:, :], in_=out_sb)
    return nc


def _get_programs():
    if "nc1" not in _CACHE:
        _CACHE["nc1"] = _build_rowsum_nc()
        _CACHE["nc2"] = _build_score_nc()
    return _CACHE["nc1"], _CACHE["nc2"]


def _run_spmd(nc, in_maps):
    from concourse.bass_utils import run_bass_kernel_spmd

    trace = bool(int(os.environ.get("CTC_TRACE", "0")))
    if trace:
        try:
            res = run_bass_kernel_spmd(
                nc, in_maps, core_ids=list(range(N_CORES)), trace=True
            )
            LAST_EXEC_TIMES.append(res.exec_time_ns)
            return res.results
        except ModuleNotFoundError:
            # NTFF hook plumbing absent in this environment; run untraced
            pass
    res = run_bass_kernel_spmd(
        nc, in_maps, core_ids=list(range(N_CORES)), trace=False
    )
    return res.results


def kernel(ctc_prob, g, c):
    import ml_dtypes

    x = np.ascontiguousarray(np.asarray(ctc_prob, dtype=np.float32))
    c_np = np.asarray(c).astype(np.int64).ravel()

    nc1, nc2 = _get_programs()

    # the device consumes x only through exp() at bf16 precision; cast on the
    # host so every core's DMA read is half the bytes
    xb = x.astype(ml_dtypes.bfloat16)
    xslices = [xb[:, k * VSLICE : (k + 1) * VSLICE] for k in range(N_CORES)]

    # ---- launch 1: partial row-sums of exp(x) per vocab slice ----
    res1 = _run_spmd(nc1, [{"x": xs} for xs in xslices])
    acc = np.zeros((TBLK, NTB), dtype=np.float32)
    for r in res1:
        acc += r["rowsum"]
    sumexp = np.empty(T, dtype=np.float32)
    for tb in range(NTB):
        sumexp[ST[tb] : ST[tb] + TBLK] = acc[:, tb]

    lse = np.log(sumexp, dtype=np.float32)               # (T,)
    blank_lp = (x[:, BLANK_COL] - lse).astype(np.float32)
    B = np.cumsum(blank_lp, dtype=np.float32)            # (T,)

    w = np.full(T, NEG_BIG, dtype=np.float32)
    w[GLEN:T] = B[GLEN - 1 : T - 1] - lse[GLEN:T]
    K = np.float32(w[GLEN:T].max())
    wv2d = np.empty((TBLK, NTB), dtype=np.float32)
    for tb in range(NTB):
        wv2d[:, tb] = w[ST[tb] : ST[tb] + TBLK] - K
    wv2d[:GLEN, 0] = NEG_BIG                 # t < GLEN contributes nothing
    dup = ST[NTB - 2] + TBLK - ST[NTB - 1]   # rows covered by both last blocks
    wv2d[:dup, NTB - 1] = NEG_BIG
    wv2d = np.ascontiguousarray(wv2d)

    # ---- launch 2: colsum[v] = sum_t exp(x[t,v] + (w[t]-K)) ----
    res2 = _run_spmd(nc2, [{"x": xs, "wv": wv2d} for xs in xslices])
    colsum = np.concatenate([r["colsum"][0] for r in res2])  # (V,)

    S = (K + np.log(colsum)).astype(np.float32)
    score = S[c_np]
    score = np.where(c_np == EOS, np.float32(B[T - 1]), score).astype(np.float32)
    return score.reshape(N_BATCH, CTC_BEAM)



# revision 5
# speedup vs baseline: 2.3945x; 2.3945x over previous
"""CTC prefix-scorer kernel for 8 Trainium2 NeuronCores (Bass/Tile).

Math: the reference's scan collapses because gamma_n_g is statically NEG_INF,
so phi_t == B[t-1] (cumsum of blank log-probs) and the n/b carries are dead.
The output is
    score[j] = logsumexp_{t=GLEN..T-1}( B[t-1] - lse[t] + ctc_prob[t, c[j]] )
    score[j] = B[T-1]                       if c[j] == EOS (==1)
    out      = score.reshape(N, ctc_beam)
which depends on j only through c[j], so we compute a per-vocab table S[v]
on-device and gather on host.

Fast path (used when c has no EOS entries): the per-step weight
u[t] = exp(B[t-1] - lse[t] - K) decays by ~|blank_lp| ~ e^-10 per step, so
only the first few t rows contribute. We process ONLY t-block [0, 128):
  Launch 1: per-core partial row-sums of exp(x[0:128, slice]) (ACT Exp with
            fused accum_out). Host combines -> lse, B, w over rows 0..127.
  Tail proof (host): w[t>=128] <= B[127] - min_row_max(x[128:]) which must be
            far below K; otherwise fall back to the full-T path.
  Launch 2: colsum[v] = sum_{t<128} exp(x[t,v] + (w[t]-K)) via ACT Exp with
            per-partition bias, then a TensorE ones-matmul over t into PSUM.
Host: S = K + log(colsum); score = S[c]; reshape. Rows t in [GLEN, 128) are
all included with their true weight (underflow to 0 where negligible), so no
window selection is needed beyond the single-block restriction.

Fallback path (EOS present, or tail proof fails): same structure over all
T=1500 rows in 12 overlapping 128-row blocks (slower, always correct).

Sharding: vocab axis (32000) split into 8 x 4000 column slices; each scan
output column only needs its own slice, and the cross-core combine (summing
128-vector row-sum partials, concatenating 4000-wide colsums) happens on host
between launches.

Hardware notes: the ACT queue has a single semaphore-wait slot, so every
Activation is arranged to depend on exactly one cross-engine event; dummy ACT
probe copies absorb additional DMA semaphores up front. DMA semaphore lanes
are collapsed to one per family (completions are FIFO per ring).
"""

import os
import numpy as np

# ---- problem constants (hardcoded per contract) ----
T = 1500
V = 32000
N_BATCH = 8
CTC_BEAM = 2048
N_CORES = 8
VSLICE = V // N_CORES  # 4000
GLEN = 9               # U-1, static in the reference
EOS = 1
BLANK_COL = V - 1
NEG_BIG = np.float32(-1e30)

TBLK = 128
NTB = 12
# full-path block starts; the last block overlaps the previous one (rows
# 1372..1499); duplicated rows are suppressed via -1e30 bias entries.
ST = [min(tb * TBLK, T - TBLK) for tb in range(NTB)]

# matmul column chunking of the 4000-wide vocab slice (PSUM bank = 512 f32)
CHUNKS = [512] * 7 + [416]

_CACHE: dict = {}
LAST_EXEC_TIMES: list = []


def _limit_dma_sem_lanes():
    """Walrus on this toolchain accepts very few semaphore waits per
    instruction; the Tile kernel-tail drain waits on every DMA sem lane the
    kernel touched. All our DMAs go through one SWDGE queue / one issuing
    HWDGE engine, whose completions are FIFO per ring, so tracking them on a
    single counting semaphore per family is sound and keeps the drain's wait
    list tiny."""
    import concourse.tile_sem_assignment as tsa

    tsa.NUM_SWDGE_GLOBAL_SEMS = 1
    tsa.NUM_HWDGE_SEMS = 1


def _patch_drain_split():
    """The kernel-tail drain waits on every logical proc the kernel used, but
    walrus here accepts a single semaphore wait per instruction. Split the
    drain's wait list across a chain of single-wait drains on the same sync
    queue (executed in order, so the union of conditions is preserved)."""
    import bass_rust
    import concourse.tile as tile_mod
    from concourse.vector_clock import ScopedClock

    if getattr(tile_mod.TileContext, "_drain_split_patched", False):
        return

    def _drain_and_barrier(self, tick_clock, wait_clock):
        # drain on the ACT ring: its queue has already observed most procs
        # (exp waits cover the DMA lanes, copies cover PE), so the wait list
        # usually collapses to a single lane wait
        drain_inst = self.nc.scalar.drain()
        wait_clock.add_sem_waits(
            drain_inst.ins, ScopedClock({None: tick_clock.global_clock})
        )
        si = drain_inst.ins.sync_info
        waits = list(si.on_wait) if si is not None else []
        if len(waits) > 1:
            drain_inst.ins.sync_info = bass_rust.SyncInfo(
                on_wait=[waits[0]], on_update=list(si.on_update)
            )
            for wt in waits[1:]:
                extra = self.nc.sync.drain()
                extra.ins.sync_info = bass_rust.SyncInfo(
                    on_wait=[wt], on_update=[]
                )

        self.nc.all_engine_barrier()
        assert self.sems is not None
        popped = self.nc._tile_sem_poison_stack.pop()
        assert popped is self._sem_poison
        self.nc.clear_and_free_semaphores(list(self.sems.allocated().values()))
        self.nc.all_engine_barrier()

    tile_mod.TileContext._drain_and_barrier = _drain_and_barrier
    tile_mod.TileContext._drain_split_patched = True


# --------------------------------------------------------------------------
# fast-path programs: single 128-row t-block
# --------------------------------------------------------------------------


def _build_rowsum1_nc():
    """x (128, VSLICE) bf16 -> rowsum (128, 1) f32 = sum_v exp(x[p, v])."""
    import concourse.bass as bass
    import concourse.tile as tile
    from concourse import mybir

    _limit_dma_sem_lanes()
    _patch_drain_split()
    nc = bass.Bass()
    x = nc.dram_tensor("x", [TBLK, VSLICE], mybir.dt.bfloat16, kind="ExternalInput")
    rowsum = nc.dram_tensor(
        "rowsum", [TBLK, 1], mybir.dt.float32, kind="ExternalOutput"
    )

    with tile.TileContext(nc) as tc:
        with tc.tile_pool(name="sing", bufs=1) as sing:
            et = sing.tile([TBLK, VSLICE], mybir.dt.bfloat16)
            h = VSLICE // 2
            # two DMA families in parallel halve time-to-data
            nc.sync.dma_start(out=et[:, :h], in_=x[:, :h])
            nc.gpsimd.dma_start(out=et[:, h:], in_=x[:, h:])
            # absorb the gpsimd-family DMA semaphore into ACT's observed
            # clock so the Exp below carries only the sync-family wait
            probe = sing.tile([TBLK, 1], mybir.dt.float32)
            nc.scalar.copy(out=probe, in_=et[:, VSLICE - 1 : VSLICE])
            partial = sing.tile([TBLK, 1], mybir.dt.float32)
            # out-of-place: an in-place Exp would WAR-conflict with the probe
            # read and pick up a second (same-engine) semaphore wait
            et2 = sing.tile([TBLK, VSLICE], mybir.dt.bfloat16)
            nc.scalar.activation(
                out=et2,
                in_=et,
                func=mybir.ActivationFunctionType.Exp,
                accum_out=partial,
            )
            nc.scalar.dma_start(out=rowsum[:, :], in_=partial)
    return nc


def _build_colsum1_nc():
    """x (128, VSLICE) bf16, wv (128, 1) f32 -> colsum (1, VSLICE) f32 with
    colsum[v] = sum_p exp(x[p, v] + wv[p])."""
    import concourse.bass as bass
    import concourse.tile as tile
    from concourse import mybir

    _limit_dma_sem_lanes()
    _patch_drain_split()
    nc = bass.Bass()
    x = nc.dram_tensor("x", [TBLK, VSLICE], mybir.dt.bfloat16, kind="ExternalInput")
    wv = nc.dram_tensor("wv", [TBLK, 1], mybir.dt.float32, kind="ExternalInput")
    colsum = nc.dram_tensor(
        "colsum", [1, VSLICE], mybir.dt.float32, kind="ExternalOutput"
    )

    with tile.TileContext(nc) as tc:
        with (
            tc.tile_pool(name="sing", bufs=1) as sing,
            tc.tile_pool(name="psp", space="PSUM", bufs=1) as psp,
        ):
            wv_sb = sing.tile([TBLK, 1], mybir.dt.float32)
            et = sing.tile([TBLK, VSLICE], mybir.dt.bfloat16)
            h = VSLICE // 2
            nc.sync.dma_start(out=et[:, :h], in_=x[:, :h])
            # wv then second x half on the same (gpsimd) ring: one counting
            # semaphore covers both in FIFO order
            nc.gpsimd.dma_start(out=wv_sb, in_=wv[:, :])
            nc.gpsimd.dma_start(out=et[:, h:], in_=x[:, h:])
            probe = sing.tile([TBLK, 1], mybir.dt.float32)
            nc.scalar.copy(out=probe, in_=et[:, VSLICE - 1 : VSLICE])
            # out-of-place: an in-place Exp would WAR-conflict with the probe
            # read and pick up a second (same-engine) semaphore wait
            et2 = sing.tile([TBLK, VSLICE], mybir.dt.bfloat16)
            nc.scalar.activation(
                out=et2,
                in_=et,
                func=mybir.ActivationFunctionType.Exp,
                bias=wv_sb,
                scale=1.0,
            )
            ones = nc.const_aps.tensor(1.0, (TBLK, 1), mybir.dt.bfloat16)
            ps = psp.tile([1, 4096], mybir.dt.float32)
            s = 0
            for cw in CHUNKS:
                nc.tensor.matmul(
                    ps[:, s : s + cw],
                    ones,
                    et2[:, s : s + cw],
                    start=True,
                    stop=True,
                )
                s += cw
            out_sb = sing.tile([1, VSLICE], mybir.dt.float32)
            nc.scalar.copy(out=out_sb, in_=ps[:, :VSLICE])
            nc.scalar.dma_start(out=colsum[:, :], in_=out_sb)
    return nc


# --------------------------------------------------------------------------
# fallback programs: all T rows in 12 overlapping blocks (always correct)
# --------------------------------------------------------------------------


def _build_rowsum_full_nc():
    """x (T, VSLICE) bf16 -> rowsum (128, NTB) f32 where
    rowsum[p, tb] = sum_v exp(x[ST[tb] + p, v])."""
    import concourse.bass as bass
    import concourse.tile as tile
    from concourse import mybir

    _limit_dma_sem_lanes()
    _patch_drain_split()
    nc = bass.Bass()
    x = nc.dram_tensor("x", [T, VSLICE], mybir.dt.bfloat16, kind="ExternalInput")
    rowsum = nc.dram_tensor(
        "rowsum", [TBLK, NTB], mybir.dt.float32, kind="ExternalOutput"
    )

    with tile.TileContext(nc) as tc:
        with tc.tile_pool(name="sing", bufs=1) as sing:
            partials = sing.tile([TBLK, NTB], mybir.dt.float32)
            for tb in range(NTB):
                et = sing.tile(
                    [TBLK, VSLICE], mybir.dt.bfloat16, name=f"et{tb}", tag=f"et{tb}"
                )
                eng = nc.sync if tb % 2 == 0 else nc.gpsimd
                eng.dma_start(out=et, in_=x[ST[tb] : ST[tb] + TBLK, :])
                nc.scalar.activation(
                    out=et,
                    in_=et,
                    func=mybir.ActivationFunctionType.Exp,
                    accum_out=partials[:, tb : tb + 1],
                )
            nc.scalar.dma_start(out=rowsum[:, :], in_=partials)
    return nc


def _build_score_full_nc():
    """x (T, VSLICE) bf16, wv (128, NTB) f32 (= w[t]-K per t-block column,
    -1e30 on invalid/duplicated t) -> colsum (1, VSLICE) f32 with
    colsum[v] = sum_t exp(x[t, v] + wv[t])."""
    import concourse.bass as bass
    import concourse.tile as tile
    from concourse import mybir

    _limit_dma_sem_lanes()
    _patch_drain_split()
    nc = bass.Bass()
    x = nc.dram_tensor("x", [T, VSLICE], mybir.dt.bfloat16, kind="ExternalInput")
    wv = nc.dram_tensor("wv", [TBLK, NTB], mybir.dt.float32, kind="ExternalInput")
    colsum = nc.dram_tensor(
        "colsum", [1, VSLICE], mybir.dt.float32, kind="ExternalOutput"
    )

    with tile.TileContext(nc) as tc:
        with (
            tc.tile_pool(name="sing", bufs=1) as sing,
            tc.tile_pool(name="psp", space="PSUM", bufs=1) as psp,
            tc.tile_pool(name="outp", bufs=1) as outp,
        ):
            wv_sb = sing.tile([TBLK, NTB], mybir.dt.float32)
            nc.gpsimd.dma_start(out=wv_sb, in_=wv[:, :])
            wv_probe = sing.tile([TBLK, 1], mybir.dt.float32)
            nc.scalar.copy(out=wv_probe, in_=wv_sb[:, 0:1])

            ones = nc.const_aps.tensor(1.0, (TBLK, 1), mybir.dt.bfloat16)

            ps = [
                psp.tile([1, 512], mybir.dt.float32, name=f"ps{ch}", tag=f"ps{ch}")
                for ch in range(len(CHUNKS))
            ]

            for tb in range(NTB):
                et = sing.tile(
                    [TBLK, VSLICE], mybir.dt.bfloat16, name=f"et{tb}", tag=f"et{tb}"
                )
                eng = nc.sync if tb % 2 == 0 else nc.gpsimd
                eng.dma_start(out=et, in_=x[ST[tb] : ST[tb] + TBLK, :])
                nc.scalar.activation(
                    out=et,
                    in_=et,
                    func=mybir.ActivationFunctionType.Exp,
                    bias=wv_sb[:, tb : tb + 1],
                    scale=1.0,
                )
                s = 0
                for ch, cw in enumerate(CHUNKS):
                    nc.tensor.matmul(
                        ps[ch][:, :cw],
                        ones,
                        et[:, s : s + cw],
                        start=(tb == 0),
                        stop=(tb == NTB - 1),
                    )
                    s += cw

            out_sb = outp.tile([1, VSLICE], mybir.dt.float32)
            s = 0
            for ch, cw in enumerate(CHUNKS):
                nc.scalar.copy(out=out_sb[:, s : s + cw], in_=ps[ch][:, :cw])
                s += cw
            nc.scalar.dma_start(out=colsum[:, :], in_=out_sb)
    return nc


def _get_program(name):
    if name not in _CACHE:
        _CACHE[name] = {
            "rs1": _build_rowsum1_nc,
            "cs1": _build_colsum1_nc,
            "rs_full": _build_rowsum_full_nc,
            "cs_full": _build_score_full_nc,
        }[name]()
    return _CACHE[name]


def _run_spmd(nc, in_maps):
    from concourse.bass_utils import run_bass_kernel_spmd

    trace = bool(int(os.environ.get("CTC_TRACE", "0")))
    if trace:
        try:
            res = run_bass_kernel_spmd(
                nc, in_maps, core_ids=list(range(N_CORES)), trace=True
            )
            LAST_EXEC_TIMES.append(res.exec_time_ns)
            return res.results
        except ModuleNotFoundError:
            # NTFF hook plumbing absent in this environment; run untraced
            pass
    res = run_bass_kernel_spmd(
        nc, in_maps, core_ids=list(range(N_CORES)), trace=False
    )
    return res.results


def _vslices(xb):
    return [
        np.ascontiguousarray(xb[:, k * VSLICE : (k + 1) * VSLICE])
        for k in range(N_CORES)
    ]


def kernel(ctc_prob, g, c):
    import ml_dtypes

    x = np.ascontiguousarray(np.asarray(ctc_prob, dtype=np.float32))
    c_np = np.asarray(c).astype(np.int64).ravel()
    has_eos = bool((c_np == EOS).any())

    if not has_eos:
        out = _kernel_fast(x, c_np, ml_dtypes)
        if out is not None:
            return out
    return _kernel_full(x, c_np, ml_dtypes)


def _kernel_fast(x, c_np, ml_dtypes):
    """Single-t-block path. Returns None if the tail proof fails."""
    xb0 = x[:TBLK].astype(ml_dtypes.bfloat16)
    xslices = _vslices(xb0)

    res1 = _run_spmd(_get_program("rs1"), [{"x": xs} for xs in xslices])
    rowsum = np.zeros(TBLK, dtype=np.float32)
    for r in res1:
        rowsum += r["rowsum"][:, 0]

    lse = np.log(rowsum, dtype=np.float32)               # (128,)
    blank_lp = (x[:TBLK, BLANK_COL] - lse).astype(np.float32)
    B = np.cumsum(blank_lp, dtype=np.float32)            # (128,)

    w = np.full(TBLK, NEG_BIG, dtype=np.float32)
    w[GLEN:] = B[GLEN - 1 : TBLK - 1] - lse[GLEN:]
    K = np.float32(w[GLEN:].max())

    # tail proof: for t >= TBLK, w[t] = B[t-1] - lse[t] <= B[TBLK-1] - lse[t]
    # (B decreasing) and lse[t] >= max_v x[t, v]; require the whole tail to
    # sit far enough below K that its summed weight is negligible.
    tail_w_bound = float(B[TBLK - 1]) - float(x[TBLK:].max(axis=1).min())
    if not (tail_w_bound < float(K) - 60.0):
        return None

    wv = (w - K).astype(np.float32)
    wv[:GLEN] = NEG_BIG
    wv2d = np.ascontiguousarray(wv.reshape(TBLK, 1))

    res2 = _run_spmd(
        _get_program("cs1"), [{"x": xs, "wv": wv2d} for xs in xslices]
    )
    colsum = np.concatenate([r["colsum"][0] for r in res2])  # (V,)

    S = (K + np.log(colsum)).astype(np.float32)
    return S[c_np].reshape(N_BATCH, CTC_BEAM).astype(np.float32)


def _kernel_full(x, c_np, ml_dtypes):
    """All-T path (handles EOS entries and adversarial weight profiles)."""
    xb = x.astype(ml_dtypes.bfloat16)
    xslices = _vslices(xb)

    res1 = _run_spmd(_get_program("rs_full"), [{"x": xs} for xs in xslices])
    acc = np.zeros((TBLK, NTB), dtype=np.float32)
    for r in res1:
        acc += r["rowsum"]
    sumexp = np.empty(T, dtype=np.float32)
    for tb in range(NTB):
        sumexp[ST[tb] : ST[tb] + TBLK] = acc[:, tb]

    lse = np.log(sumexp, dtype=np.float32)               # (T,)
    blank_lp = (x[:, BLANK_COL] - lse).astype(np.float32)
    B = np.cumsum(blank_lp, dtype=np.float32)            # (T,)

    w = np.full(T, NEG_BIG, dtype=np.float32)
    w[GLEN:T] = B[GLEN - 1 : T - 1] - lse[GLEN:T]
    K = np.float32(w[GLEN:T].max())
    wv2d = np.empty((TBLK, NTB), dtype=np.float32)
    for tb in range(NTB):
        wv2d[:, tb] = w[ST[tb] : ST[tb] + TBLK] - K
    wv2d[:GLEN, 0] = NEG_BIG                 # t < GLEN contributes nothing
    dup = ST[NTB - 2] + TBLK - ST[NTB - 1]   # rows covered by both last blocks
    wv2d[:dup, NTB - 1] = NEG_BIG
    wv2d = np.ascontiguousarray(wv2d)

    res2 = _run_spmd(
        _get_program("cs_full"), [{"x": xs, "wv": wv2d} for xs in xslices]
    )
    colsum = np.concatenate([r["colsum"][0] for r in res2])  # (V,)

    S = (K + np.log(colsum)).astype(np.float32)
    score = S[c_np]
    score = np.where(c_np == EOS, np.float32(B[T - 1]), score).astype(np.float32)
    return score.reshape(N_BATCH, CTC_BEAM)
